# revision 1
# baseline (speedup 1.0000x reference)
"""Trainium2 Bass kernel for the spiking autoencoder (nn_AE_spikes).

Algorithm
---------
reference: 16 timesteps of integrate-and-fire over 4 layers (784-128-128-128-784).
Identities used:
  * encoder: cumulative spike count f_t = sum_{tau<=t} s0_tau = floor(t*x),
    computed fresh each step as rne(t*x - 0.5) via f32->i16 cast (TRN2 RNE).
  * every hidden layer i in "w-form": PSUM accumulates G_i = sum_t W_i @ s_{i-1,t}
    (for layer 1, G_1 = W_1 @ f_t directly since f is cumulative);
    spike:  s_i = (G_i - (1 - t*b_i)) >= wneg_i,  then wneg_i += s_i.
  * layer 4 is linear in the spikes: out = (W4/16) @ (sum_t s3_t) + b4 --
    one matmul per batch tile.
  * all matmuls in bf16 with hi/lo-split weights (W = bf16(W) + bf16(W-bf16(W)));
    every matmul rhs (f_t, spikes, spike-sums) is integer-valued and exact in
    bf16, so the only error is the double-rounded weight (~4e-6 relative).

Engine placement (HW-calibrated): the f32->i16 floor runs on ScalarE (one
big activation), the i16->bf16 convert on VectorE (4x mode), spikes (fused
scalar_tensor_tensor) and wneg updates on VectorE; GPSIMD is kept out of
the hot path (its converting copies run at ~4 cyc/elem).  Batch tiles are
R=1024 wide so each matmul weight-load serves two N=512 matmuls and
per-op/semaphore overheads amortize.

Layout: feature-major on device ([feature, batch] tiles); the host
pre-transposes `features` (batch-sharded over 8 cores, pure data parallel)
and post-transposes the output.
"""

import numpy as np
import ml_dtypes

import concourse.bass as bass
import concourse.mybir as mybir
from concourse import bacc
from concourse.tile import TileContext, add_dep_helper
from concourse.bass_utils import run_bass_kernel_spmd

F32 = mybir.dt.float32
BF16 = mybir.dt.bfloat16
I32 = mybir.dt.int32
I16 = mybir.dt.int16
Alu = mybir.AluOpType
ActFn = mybir.ActivationFunctionType

N_CORES = 8
BATCH = 32768
ROWS_PER_CORE = BATCH // N_CORES     # 4096
R = 1024                             # batch rows per tile
RH = 512                             # matmul half-width (PSUM bank limit)
D_IN = 784
NCHUNK = 7                           # 784 = 7 * 112 ; padded 896 = 7 * 128
D_PAD = NCHUNK * 128                 # 896
H = 128
T = 16
N_TILES = ROWS_PER_CORE // R         # 4

# how many of the 7 ff chunks the Scalar engine converts (rest on VectorE)
FF_ACT_CHUNKS = 2

# weight const tile (bf16) free-offset layout
_OFF_W1H = 0                          # [128, 896]
_OFF_W1L = _OFF_W1H + D_PAD           # [128, 896]
_OFF_W2H = _OFF_W1L + D_PAD           # [128, 128]
_OFF_W2L = _OFF_W2H + H
_OFF_W3H = _OFF_W2L + H
_OFF_W3L = _OFF_W3H + H
_OFF_W4H = _OFF_W3L + H               # [128, 784]
_OFF_W4L = _OFF_W4H + D_IN
_OFF_ID = _OFF_W4L + D_IN             # [128, 128] (unused, kept for layout)
CWW = _OFF_ID + H                     # 4000
# scalar const tile (f32) free-offset layout
_OFF_THR1 = 0                         # [128, 16]
_OFF_THR2 = _OFF_THR1 + T
_OFF_THR3 = _OFF_THR2 + T
_OFF_B4C = _OFF_THR3 + T              # [128, 7]
_OFF_NEGHALF = _OFF_B4C + NCHUNK      # [128, 1] = -0.5
CW = _OFF_NEGHALF + 1                 # 56


def _build_nc(bench_loop=False):
    nc = bacc.Bacc("TRN2", target_bir_lowering=False, debug=False,
                   enable_asserts=False, num_devices=N_CORES)
    if bench_loop:
        niter_ext = nc.declare_dram_parameter("niter", [1, 1], I32,
                                              isOutput=False)
    constw_ext = nc.declare_dram_parameter("constw", [128, CWW], BF16,
                                           isOutput=False)
    consts_ext = nc.declare_dram_parameter("consts", [128, CW], F32,
                                           isOutput=False)
    xT_ext = nc.declare_dram_parameter("xT", [D_PAD, ROWS_PER_CORE], F32,
                                       isOutput=False)
    outT_ext = nc.declare_dram_parameter("outT", [D_IN, ROWS_PER_CORE], F32,
                                         isOutput=True)

    with TileContext(nc) as tc:
        with (
            tc.tile_pool(name="const", bufs=1) as cpool,
            tc.tile_pool(name="io", bufs=2) as iopool,
            tc.tile_pool(name="ost", bufs=2) as ostpool,
            tc.tile_pool(name="work", bufs=2) as wpool,
            tc.tile_pool(name="fip", bufs=2) as fipool,
            tc.tile_pool(name="state", bufs=2) as spool,
            tc.tile_pool(name="ps_g", bufs=2, space="PSUM") as pg,
            tc.tile_pool(name="ps_h", bufs=1, space="PSUM") as ph,
        ):
            constw = cpool.tile([128, CWW], BF16)
            nc.sync.dma_start(out=constw, in_=constw_ext[:, :])
            consts = cpool.tile([128, CW], F32)
            nc.sync.dma_start(out=consts, in_=consts_ext[:, :])

            def w1h(c):
                return constw[:, _OFF_W1H + c * 128:_OFF_W1H + (c + 1) * 128]

            def w1l(c):
                return constw[:, _OFF_W1L + c * 128:_OFF_W1L + (c + 1) * 128]

            w2h = constw[:, _OFF_W2H:_OFF_W2H + H]
            w2l = constw[:, _OFF_W2L:_OFF_W2L + H]
            w3h = constw[:, _OFF_W3H:_OFF_W3H + H]
            w3l = constw[:, _OFF_W3L:_OFF_W3L + H]
            w4h = constw[:, _OFF_W4H:_OFF_W4H + D_IN]
            w4l = constw[:, _OFF_W4L:_OFF_W4L + D_IN]
            thr1 = consts[:, _OFF_THR1:_OFF_THR1 + T]
            thr2 = consts[:, _OFF_THR2:_OFF_THR2 + T]
            thr3 = consts[:, _OFF_THR3:_OFF_THR3 + T]
            b4c = consts[:, _OFF_B4C:_OFF_B4C + NCHUNK]
            neghalf = consts[:, _OFF_NEGHALF:_OFF_NEGHALF + 1]

            # PE instruction-order pinning: ldweights loads the PE array
            # once per weight; the matmuls that use it must not be reordered
            # around it, so every PE instruction is chained with a
            # scheduling-only dependency edge (runtime cost: none).
            _pe_prev = [None]

            def _pe(bi):
                if _pe_prev[0] is not None:
                    add_dep_helper(bi.ins, _pe_prev[0], sync=False,
                                   reason="pe-order")
                _pe_prev[0] = bi.ins
                return bi

            def mm_group(w, jobs):
                for out_ap, rhs_ap, st, sp in jobs:
                    nc.tensor.matmul(out_ap, w, rhs_ap, start=st,
                                     stop=sp, skip_group_check=True)

            # PE primer: absorb the const-DMA wait on the PE clock early.
            prime = pg.tile([128, R], F32, name="prime", tag="g1")
            nc.tensor.matmul(prime[0:1, 0:2], constw[:, 0:1],
                             constw[:, 0:2], start=True, stop=True)

            from contextlib import ExitStack as _ES
            _stk = _ES()
            if bench_loop:
                nt = cpool.tile([1, 1], I32, name="nt")
                nc.sync.dma_start(out=nt, in_=niter_ext[:, :])
                regs = []
                for ename in mybir.ALL_ENGINES:
                    eng = nc.engines[ename]
                    r = eng.alloc_register(f"niter_{ename.name}")
                    eng.reg_load(r, nt[0:1, 0:1])
                    regs.append(r)
                nloop = nc.snap(bass.RegisterHandles(regs), donate=True,
                                min_val=0, max_val=1 << 20)
                _stk.enter_context(
                    tc.For_i(0, nloop,
                             hint_engines=tuple(mybir.ALL_ENGINES)))

            for bt in range(N_TILES):
                r0 = bt * R
                x = iopool.tile([128, NCHUNK * R], F32, name="x")
                nc.sync.dma_start(
                    out=x.rearrange("p (c r) -> p c r", r=R),
                    in_=xT_ext[:, r0:r0 + R].rearrange("(c p) r -> p c r",
                                                       p=128),
                )
                wneg1 = spool.tile([128, R], BF16, name="wneg1")
                nc.vector.memset(wneg1, 0.0)
                wneg2 = spool.tile([128, R], BF16, name="wneg2")
                nc.vector.memset(wneg2, 0.0)
                wneg3 = spool.tile([128, R], BF16, name="wneg3")
                nc.vector.memset(wneg3, 0.0)
                s3sum = spool.tile([128, R], BF16, name="s3sum")
                nc.vector.memset(s3sum, 0.0)
                G2 = ph.tile([128, R], F32, name="g2acc")
                G3 = ph.tile([128, R], F32, name="g3acc")

                for tp in range(T // 2):
                    t0 = 2 * tp + 1          # pair of steps (t0, t0+1)
                    ffs = {}
                    g1s = {}
                    for t in (t0, t0 + 1):
                        # f_t = rne(t*x - 0.5) = floor(t*x), int16 (ACT pass)
                        fi = fipool.tile([128, NCHUNK * R], I16, name="fi")
                        nc.scalar.activation(fi, x, ActFn.Identity,
                                             bias=neghalf, scale=float(t))
                        # i16 -> bf16 (exact): DVE 4x mode + some ACT chunks
                        ff = wpool.tile([128, NCHUNK * R], BF16, name="ff")
                        na = FF_ACT_CHUNKS * R
                        nc.scalar.copy(ff[:, :na], fi[:, :na])
                        nc.vector.tensor_copy(ff[:, na:], fi[:, na:])
                        ffs[t] = ff
                        g1s[t] = pg.tile([128, R], F32, name="g1", tag="g1")
                    # L1 for both steps with each weight loaded once
                    for c in range(NCHUNK):
                        for wi, w in enumerate((w1h(c), w1l(c))):
                            mm_group(w, [
                                (g1s[t][:, hx * RH:(hx + 1) * RH],
                                 ffs[t][:, c * R + hx * RH:
                                        c * R + hx * RH + RH],
                                 (c == 0 and wi == 0),
                                 (c == NCHUNK - 1 and wi == 1))
                                for t in (t0, t0 + 1) for hx in range(2)])

                    for t in (t0, t0 + 1):
                        s1 = wpool.tile([128, R], BF16, name="s1")
                        nc.vector.scalar_tensor_tensor(
                            s1, g1s[t], thr1[:, t - 1:t], wneg1,
                            op0=Alu.subtract, op1=Alu.is_ge)
                        nc.vector.tensor_tensor(wneg1, wneg1, s1, Alu.add)

                        for wi, w in enumerate((w2h, w2l)):
                            mm_group(w, [
                                (G2[:, hx * RH:(hx + 1) * RH],
                                 s1[:, hx * RH:(hx + 1) * RH],
                                 (t == 1 and wi == 0),
                                 (t == T and wi == 1))
                                for hx in range(2)])
                        s2 = wpool.tile([128, R], BF16, name="s2")
                        nc.vector.scalar_tensor_tensor(
                            s2, G2, thr2[:, t - 1:t], wneg2,
                            op0=Alu.subtract, op1=Alu.is_ge)
                        nc.vector.tensor_tensor(wneg2, wneg2, s2, Alu.add)

                        for wi, w in enumerate((w3h, w3l)):
                            mm_group(w, [
                                (G3[:, hx * RH:(hx + 1) * RH],
                                 s2[:, hx * RH:(hx + 1) * RH],
                                 (t == 1 and wi == 0),
                                 (t == T and wi == 1))
                                for hx in range(2)])
                        s3 = wpool.tile([128, R], BF16, name="s3")
                        nc.vector.scalar_tensor_tensor(
                            s3, G3, thr3[:, t - 1:t], wneg3,
                            op0=Alu.subtract, op1=Alu.is_ge)
                        nc.vector.tensor_tensor(wneg3, wneg3, s3, Alu.add)
                        nc.gpsimd.tensor_tensor(s3sum, s3sum, s3, Alu.add)

                outstage = ostpool.tile([128, NCHUNK * R], F32,
                                        name="outstage")
                for c in range(NCHUNK):
                    l4 = pg.tile([112, R], F32, name="l4", tag="g1")
                    for wi, w in enumerate((w4h, w4l)):
                        mm_group(w[:, c * 112:(c + 1) * 112], [
                            (l4[:, hx * RH:(hx + 1) * RH],
                             s3sum[:, hx * RH:(hx + 1) * RH],
                             (wi == 0), (wi == 1))
                            for hx in range(2)])
                    nc.scalar.activation(
                        outstage[0:112, c * R:(c + 1) * R], l4,
                        ActFn.Identity, bias=b4c[0:112, c:c + 1], scale=1.0)
                nc.sync.dma_start(
                    out=outT_ext[:, r0:r0 + R].rearrange("(c p) r -> p c r",
                                                         p=112),
                    in_=outstage[0:112, :].rearrange("p (c r) -> p c r", r=R),
                )
            _stk.close()

    nc.compile()
    return nc


def _bf(x):
    return x.astype(ml_dtypes.bfloat16).astype(np.float32)


def _prep_consts(W1, b1, W2, b2, W3, b3, W4, b4):
    constw = np.zeros((128, CWW), np.float32)

    def pack_w1(W, off):
        Wp = np.zeros((D_PAD, H), np.float32)
        Wp[:D_IN] = W.T
        constw[:, off:off + D_PAD] = (
            Wp.reshape(NCHUNK, 128, H).transpose(1, 0, 2).reshape(128, D_PAD))

    W1h = _bf(W1)
    pack_w1(W1h, _OFF_W1H)
    pack_w1(W1 - W1h, _OFF_W1L)
    W2h = _bf(W2)
    constw[:, _OFF_W2H:_OFF_W2H + H] = W2h.T
    constw[:, _OFF_W2L:_OFF_W2L + H] = (W2 - W2h).T
    W3h = _bf(W3)
    constw[:, _OFF_W3H:_OFF_W3H + H] = W3h.T
    constw[:, _OFF_W3L:_OFF_W3L + H] = (W3 - W3h).T
    W4s = W4 / np.float32(T)
    W4h = _bf(W4s)
    constw[:, _OFF_W4H:_OFF_W4H + D_IN] = W4h.T
    constw[:, _OFF_W4L:_OFF_W4L + D_IN] = (W4s - W4h).T
    constw[:, _OFF_ID:_OFF_ID + H] = np.eye(H, dtype=np.float32)

    consts = np.zeros((128, CW), np.float32)
    for t in range(1, T + 1):
        consts[:, _OFF_THR1 + t - 1] = np.float32(1.0) - np.float32(t) * b1
        consts[:, _OFF_THR2 + t - 1] = np.float32(1.0) - np.float32(t) * b2
        consts[:, _OFF_THR3 + t - 1] = np.float32(1.0) - np.float32(t) * b3
    consts[:112, _OFF_B4C:_OFF_B4C + NCHUNK] = b4.reshape(NCHUNK, 112).T
    consts[:, _OFF_NEGHALF] = -0.5
    return constw.astype(ml_dtypes.bfloat16), consts


_NC_CACHE = {}


def _get_nc():
    if "nc" not in _NC_CACHE:
        _NC_CACHE["nc"] = _build_nc()
    return _NC_CACHE["nc"]


def _run(inputs, trace=False):
    features = np.asarray(inputs["features"], np.float32)
    constw, consts = _prep_consts(
        np.asarray(inputs["W1"], np.float32), np.asarray(inputs["b1"], np.float32),
        np.asarray(inputs["W2"], np.float32), np.asarray(inputs["b2"], np.float32),
        np.asarray(inputs["W3"], np.float32), np.asarray(inputs["b3"], np.float32),
        np.asarray(inputs["W4"], np.float32), np.asarray(inputs["b4"], np.float32))

    in_maps = []
    for c in range(N_CORES):
        rows = features[c * ROWS_PER_CORE:(c + 1) * ROWS_PER_CORE]
        xT = np.zeros((D_PAD, ROWS_PER_CORE), np.float32)
        xT[:D_IN] = rows.T
        in_maps.append({"constw": constw, "consts": consts, "xT": xT})

    nc = _get_nc()
    try:
        res = run_bass_kernel_spmd(nc, in_maps, core_ids=list(range(N_CORES)),
                                   trace=trace)
    except Exception:
        # first execution of a freshly compiled NEFF occasionally trips a
        # transient NRT_EXEC_UNIT_UNRECOVERABLE on this fleet; retry once
        res = run_bass_kernel_spmd(nc, in_maps, core_ids=list(range(N_CORES)),
                                   trace=trace)
    out = np.empty((BATCH, D_IN), np.float32)
    for c in range(N_CORES):
        out[c * ROWS_PER_CORE:(c + 1) * ROWS_PER_CORE] = \
            res.results[c]["outT"].T
    return out, res


def kernel(**inputs) -> np.ndarray:
    out, _ = _run(inputs)
    return out

